# revision 1
# baseline (speedup 1.0000x reference)
"""Trainium2 Bass kernel for nn_Encoder (embedding -> LSTM scan with EOS
state-freezing, returns final (c, h) carry).

Key structural fact: the reference's EOS flag for a sequence is set from
``x[:, EOS_ID].astype(bool)`` where ``x`` is the *float* embedding row of the
current token.  A sequence's state therefore freezes permanently after the
first step whose token embedding has a nonzero feature at column EOS_ID.  The
host computes the exact number of scan steps ``T`` after which every
sequence is frozen (for randn-filled embeddings T == 1 with probability 1)
and the device only has to run those T steps.  For T == 1 the step
simplifies exactly (no approximation): h0 == c0 == 0, so the Wh matmul and
the forget gate contribute exactly nothing:

    gates = x0 @ Wx + b
    c = sigmoid(gates_i) * tanh(gates_g)
    h = sigmoid(gates_o) * tanh(c)

Sharding: the hidden dimension (and with it the i/g/o gate columns of Wx) is
split across the 8 cores, 64 hidden units each.  Each core gathers the 64
first-token embedding rows from the (replicated) table, computes its
[64 hidden x 64 batch] chunk of c and h in transposed layout, and the host
concatenates/transposes the chunks into the full [64, 512] outputs.
"""

import numpy as np

B, S, V, E, H = 64, 512, 32000, 512, 512
EOS_ID = 1
N_CORES = 8
HSH = H // N_CORES  # hidden slice per core: 64
KCH = E // 128      # contraction chunks: 4

_cache = {}


def _sigmoid(x):
    return 1.0 / (1.0 + np.exp(-x))


def _lstm_numpy(inputs, embedding, Wx, Wh, b):
    """Faithful float32 fallback for the (probability ~0) case where not all
    sequences hit EOS on the first step."""
    Bn = inputs.shape[0]
    c = np.zeros((Bn, H), np.float32)
    h = np.zeros((Bn, H), np.float32)
    eos = np.zeros((Bn,), bool)
    for t in range(inputs.shape[1]):
        x = embedding[inputs[:, t]]
        g = x @ Wx + h @ Wh + b
        gi, gf, gg, go = np.split(g, 4, axis=1)
        new_c = _sigmoid(gf) * c + _sigmoid(gi) * np.tanh(gg)
        new_h = _sigmoid(go) * np.tanh(new_c)
        keep = eos[:, None]
        c = np.where(keep, c, new_c)
        h = np.where(keep, h, new_h)
        eos |= embedding[inputs[:, t], EOS_ID] != 0
        if eos.all():
            break
    return c, h


def _build_t1_program():
    """One-step LSTM cell, gate-column sharded, transposed layout."""
    import concourse.bacc as bacc
    import concourse.mybir as mybir
    import concourse.tile as tile

    f32 = mybir.dt.float32
    nc = bacc.Bacc("TRN2", target_bir_lowering=False, debug=False,
                   num_devices=N_CORES)

    emb = nc.declare_dram_parameter("emb", [V, E], f32, isOutput=False)
    # Wx gate columns for this core, K-chunk major: [KCH, 128, 3*HSH]
    wx = nc.declare_dram_parameter("wx", [KCH, 128, 3 * HSH], f32, isOutput=False)
    # bias for this core's i/g/o slices, batch-partition broadcastless: [64, 3]
    bg = nc.declare_dram_parameter("bg", [HSH, 3], f32, isOutput=False)
    tok = nc.declare_dram_parameter("tok", [B, 1], mybir.dt.int32, isOutput=False)
    iden = nc.declare_dram_parameter("iden", [B, B], f32, isOutput=False)
    ct = nc.declare_dram_parameter("ct", [HSH, B], f32, isOutput=True)
    ht = nc.declare_dram_parameter("ht", [HSH, B], f32, isOutput=True)

    with tile.TileContext(nc) as tc:
        with (
            tc.tile_pool(name="sbuf", bufs=1) as sb,
            tc.tile_pool(name="psum", bufs=1, space="PSUM") as ps,
        ):
            # Big weight DMA first so it overlaps the gather+transpose chain.
            wx_sb = sb.tile([128, KCH, 3 * HSH], f32, tag="wx")
            nc.sync.dma_start(wx_sb[:], wx.ap().rearrange("c p m -> p c m"))

            bias_sb = sb.tile([HSH, 3], f32, tag="bias")
            nc.sync.dma_start(bias_sb[:], bg[:])

            iden_sb = sb.tile([B, B], f32, tag="iden")
            nc.sync.dma_start(iden_sb[:], iden[:])

            tok_sb = sb.tile([B, 1], mybir.dt.int32, tag="tok")
            nc.sync.dma_start(tok_sb[:], tok[:])

            # Gather the 64 first-token embedding rows: [B, E]
            x_sb = sb.tile([B, E], f32, tag="x")
            import concourse.bass as bass
            nc.gpsimd.indirect_dma_start(
                out=x_sb[:],
                out_offset=None,
                in_=emb[:],
                in_offset=bass.IndirectOffsetOnAxis(ap=tok_sb[:, :1], axis=0),
            )

            # Transpose to [E, B] in 4 chunks of 128 partitions.
            xt_sb = sb.tile([128, KCH, B], f32, tag="xt")
            for c in range(KCH):
                tp = ps.tile([128, B], f32, tag=f"tp{c}")
                nc.tensor.transpose(tp[:], x_sb[:, c * 128:(c + 1) * 128],
                                    iden_sb[:])
                nc.vector.tensor_copy(xt_sb[:, c, :], tp[:])

            # gates.T chunks: psum[g] = sum_c wx[:, c, g].T @ xt[:, c, :]
            gate_ps = []
            for g in range(3):
                p = ps.tile([HSH, B], f32, tag=f"gate{g}")
                gate_ps.append(p)
                for c in range(KCH):
                    nc.tensor.matmul(
                        p[:],
                        lhsT=wx_sb[:, c, g * HSH:(g + 1) * HSH],
                        rhs=xt_sb[:, c, :],
                        start=(c == 0),
                        stop=(c == KCH - 1),
                    )

            Act = mybir.ActivationFunctionType
            sig_i = sb.tile([HSH, B], f32, tag="sig_i")
            nc.scalar.activation(sig_i[:], gate_ps[0][:], Act.Sigmoid,
                                 bias=bias_sb[:, 0:1])
            tanh_g = sb.tile([HSH, B], f32, tag="tanh_g")
            nc.scalar.activation(tanh_g[:], gate_ps[1][:], Act.Tanh,
                                 bias=bias_sb[:, 1:2])
            c_sb = sb.tile([HSH, B], f32, tag="c")
            nc.vector.tensor_mul(c_sb[:], sig_i[:], tanh_g[:])

            sig_o = sb.tile([HSH, B], f32, tag="sig_o")
            nc.scalar.activation(sig_o[:], gate_ps[2][:], Act.Sigmoid,
                                 bias=bias_sb[:, 2:3])
            tanh_c = sb.tile([HSH, B], f32, tag="tanh_c")
            nc.scalar.activation(tanh_c[:], c_sb[:], Act.Tanh)
            h_sb = sb.tile([HSH, B], f32, tag="h")
            nc.vector.tensor_mul(h_sb[:], sig_o[:], tanh_c[:])

            nc.sync.dma_start(ct[:], c_sb[:])
            nc.sync.dma_start(ht[:], h_sb[:])

    nc.compile()
    return nc


def _run_t1(inputs, embedding, Wx, b):
    from concourse.bass_utils import run_bass_kernel_spmd

    if "t1" not in _cache:
        _cache["t1"] = _build_t1_program()
    nc = _cache["t1"]

    tok = np.ascontiguousarray(inputs[:, 0].astype(np.int32).reshape(B, 1))
    iden = np.eye(B, dtype=np.float32)
    in_maps = []
    for k in range(N_CORES):
        sl = slice(k * HSH, (k + 1) * HSH)
        # gate columns of Wx for this core: i, g, o slices (f unused: c0 == 0)
        wx_k = np.concatenate(
            [Wx[:, 0 * H:1 * H][:, sl], Wx[:, 2 * H:3 * H][:, sl],
             Wx[:, 3 * H:4 * H][:, sl]], axis=1)
        wx_k = np.ascontiguousarray(wx_k.reshape(KCH, 128, 3 * HSH))
        bg_k = np.stack(
            [b[0 * H:1 * H][sl], b[2 * H:3 * H][sl], b[3 * H:4 * H][sl]],
            axis=1).astype(np.float32)
        bg_k = np.ascontiguousarray(bg_k)
        in_maps.append({
            "emb": embedding, "wx": wx_k, "bg": bg_k, "tok": tok, "iden": iden,
        })

    res = run_bass_kernel_spmd(nc, in_maps, core_ids=list(range(N_CORES)))
    c = np.empty((B, H), np.float32)
    h = np.empty((B, H), np.float32)
    for k in range(N_CORES):
        sl = slice(k * HSH, (k + 1) * HSH)
        c[:, sl] = res.results[k]["ct"].T
        h[:, sl] = res.results[k]["ht"].T
    return c, h


def kernel(inputs, embedding, Wx, Wh, b):
    inputs = np.asarray(inputs)
    embedding = np.asarray(embedding, dtype=np.float32)
    Wx = np.asarray(Wx, dtype=np.float32)
    Wh = np.asarray(Wh, dtype=np.float32)
    b = np.asarray(b, dtype=np.float32)

    # Exact host-side computation of how many scan steps can change state:
    # sequence bb freezes forever after its first step with
    # embedding[token, EOS_ID] != 0.
    eos = np.zeros((inputs.shape[0],), bool)
    T = 0
    for t in range(inputs.shape[1]):
        eos |= embedding[inputs[:, t], EOS_ID] != 0
        T = t + 1
        if eos.all():
            break

    if T == 1:
        return _run_t1(inputs, embedding, Wx, b)
    # Probability-zero fallback (an embedding value exactly 0.0 at EOS_ID).
    return _lstm_numpy(inputs, embedding, Wx, Wh, b)


# revision 3
# speedup vs baseline: 1.0294x; 1.0294x over previous
"""Trainium2 Bass kernel for nn_Encoder (embedding -> LSTM scan with EOS
state-freezing, returns final (c, h) carry).

Key structural fact: the reference's EOS flag for a sequence is set from
``x[:, EOS_ID].astype(bool)`` where ``x`` is the *float* embedding row of the
current token.  A sequence's state therefore freezes permanently after the
first step whose token embedding has a nonzero feature at column EOS_ID.  The
host computes the exact number of scan steps ``T`` after which every
sequence is frozen (for randn-filled embeddings T == 1 with probability 1)
and the device only has to run those T steps.  For T == 1 the step
simplifies exactly (no approximation): h0 == c0 == 0, so the Wh matmul and
the forget gate contribute exactly nothing:

    gates = x0 @ Wx + b
    c = sigmoid(gates_i) * tanh(gates_g)
    h = sigmoid(gates_o) * tanh(c)

Sharding: the hidden dimension (and with it the i/g/o gate columns of Wx) is
split across the 8 cores, 64 hidden units each.  Each core gathers the 64
first-token embedding rows from the (replicated) table, computes its
[64 batch x 64 hidden] chunk of c and h, and the host concatenates the
chunks into the full [64, 512] outputs.

Device program per core (batch-major gate layout):
  aux DMA (identity + token ids)        [scalar HWDGE queue]
  bias/ones DMA                         [scalar HWDGE queue]
  Wx gate-column shard DMA              [sync HWDGE queue]
  indirect gather of 64 embedding rows, split in two column halves [SWDGE]
  4x PE transpose -> x^T chunks [128, 64]
  gates[64B, 192] = ones^T @ bias  +  sum_c x^T_c^T @ Wx_c   (PSUM accum)
  sigmoid/tanh/mul -> c rows [0:64], h rows [64:128] of one SBUF tile
  one output DMA [128, 64]
"""

import numpy as np

B, S, V, E, H = 64, 512, 32000, 512, 512
EOS_ID = 1
N_CORES = 8
HSH = H // N_CORES  # hidden slice per core: 64
G3 = 3 * HSH        # i/g/o gate columns per core: 192
KCH = E // 128      # contraction chunks: 4

_cache = {}


def _sigmoid(x):
    return 1.0 / (1.0 + np.exp(-x))


def _lstm_numpy(inputs, embedding, Wx, Wh, b):
    """Faithful float32 fallback for the (probability ~0) case where not all
    sequences hit EOS on the first step."""
    Bn = inputs.shape[0]
    c = np.zeros((Bn, H), np.float32)
    h = np.zeros((Bn, H), np.float32)
    eos = np.zeros((Bn,), bool)
    for t in range(inputs.shape[1]):
        x = embedding[inputs[:, t]]
        g = x @ Wx + h @ Wh + b
        gi, gf, gg, go = np.split(g, 4, axis=1)
        new_c = _sigmoid(gf) * c + _sigmoid(gi) * np.tanh(gg)
        new_h = _sigmoid(go) * np.tanh(new_c)
        keep = eos[:, None]
        c = np.where(keep, c, new_c)
        h = np.where(keep, h, new_h)
        eos |= embedding[inputs[:, t], EOS_ID] != 0
        if eos.all():
            break
    return c, h


def _build_t1_program():
    """One-step LSTM cell, gate-column sharded, batch-major gates."""
    import concourse.bacc as bacc
    import concourse.bass as bass
    import concourse.mybir as mybir
    import concourse.tile as tile

    f32 = mybir.dt.float32
    nc = bacc.Bacc("TRN2", target_bir_lowering=False, debug=False,
                   num_devices=N_CORES)

    emb = nc.declare_dram_parameter("emb", [V, E], f32, isOutput=False)
    # Wx gate columns for this core, K-chunk major: [KCH, 128, 192]
    wx = nc.declare_dram_parameter("wx", [KCH, 128, G3], f32, isOutput=False)
    # identity (cols 0:64) + first-token ids as int32 bit pattern (col 64)
    aux = nc.declare_dram_parameter("aux", [B, B + 1], f32, isOutput=False)
    # ones (cols 0:64) + i/g/o bias slices (cols 64:256), single row
    bgp = nc.declare_dram_parameter("bgp", [1, B + G3], f32, isOutput=False)
    # rows 0:64 = c chunk [batch, hid], rows 64:128 = h chunk
    y = nc.declare_dram_parameter("y", [2 * B, HSH], f32, isOutput=True)

    with tile.TileContext(nc) as tc:
        with (
            tc.tile_pool(name="sbuf", bufs=1) as sb,
            tc.tile_pool(name="psum", bufs=1, space="PSUM") as ps,
        ):
            # Critical path first: aux gates the gather and the transposes.
            aux_sb = sb.tile([B, B + 1], f32, tag="aux")
            nc.scalar.dma_start(aux_sb[:], aux[:])
            bgp_sb = sb.tile([1, B + G3], f32, tag="bgp")
            nc.scalar.dma_start(bgp_sb[:], bgp[:])
            wx_sb = sb.tile([128, KCH, G3], f32, tag="wx")
            nc.sync.dma_start(wx_sb[:], wx.ap().rearrange("c p m -> p c m"))

            tok_ap = aux_sb[:, B:B + 1].bitcast(mybir.dt.int32)
            iden_ap = aux_sb[:, 0:B]

            # Gather the 64 first-token embedding rows, split in two column
            # halves so the first transposes can start while the second half
            # is still in flight.
            x_sb = sb.tile([B, E], f32, tag="x")
            half = E // 2
            for hh in range(2):
                nc.gpsimd.indirect_dma_start(
                    out=x_sb[:, hh * half:(hh + 1) * half],
                    out_offset=None,
                    in_=emb[:],
                    in_offset=bass.IndirectOffsetOnAxis(ap=tok_ap, axis=0),
                    element_offset=hh * half,
                )

            # Transpose to [E, B] in 4 chunks of 128 partitions.
            xt_sb = sb.tile([128, KCH, B], f32, tag="xt")
            for c in range(KCH):
                tp = ps.tile([128, B], f32, tag=f"tp{c}")
                nc.tensor.transpose(tp[:], x_sb[:, c * 128:(c + 1) * 128],
                                    iden_ap)
                nc.vector.tensor_copy(xt_sb[:, c, :], tp[:])

            # gates [64 batch, 192] = 1^T @ bias + sum_c xt_c^T @ wx_c
            gp = ps.tile([B, G3], f32, tag="gates")
            nc.tensor.matmul(gp[:], lhsT=bgp_sb[0:1, 0:B],
                             rhs=bgp_sb[0:1, B:B + G3], start=True, stop=False)
            for c in range(KCH):
                nc.tensor.matmul(gp[:], lhsT=xt_sb[:, c, :],
                                 rhs=wx_sb[:, c, :], start=False,
                                 stop=(c == KCH - 1))

            Act = mybir.ActivationFunctionType
            out_sb = sb.tile([2 * B, HSH], f32, tag="out")
            sig_i = sb.tile([B, HSH], f32, tag="sig_i")
            nc.scalar.activation(sig_i[:], gp[:, 0:HSH], Act.Sigmoid)
            tanh_g = sb.tile([B, HSH], f32, tag="tanh_g")
            nc.scalar.activation(tanh_g[:], gp[:, HSH:2 * HSH], Act.Tanh)
            # c chunk -> rows 0:64 of the output tile
            nc.vector.tensor_mul(out_sb[0:B, :], sig_i[:], tanh_g[:])

            # h = sigmoid(o) * tanh(c), computed in partitions 64:128 so the
            # output leaves in a single DMA.
            so_tc = sb.tile([2 * B, HSH], f32, tag="so_tc")
            nc.scalar.activation(so_tc[B:2 * B, :], gp[:, 2 * HSH:G3],
                                 Act.Sigmoid)
            tc2 = sb.tile([2 * B, HSH], f32, tag="tc2")
            nc.scalar.activation(tc2[B:2 * B, :], out_sb[0:B, :], Act.Tanh)
            nc.vector.tensor_mul(out_sb[B:2 * B, :], so_tc[B:2 * B, :],
                                 tc2[B:2 * B, :])

            nc.sync.dma_start(y[:], out_sb[:])

    nc.compile()
    return nc


def _make_in_maps(inputs, embedding, Wx, b):
    tok = inputs[:, 0].astype(np.int32)
    aux = np.zeros((B, B + 1), np.float32)
    aux[:, :B] = np.eye(B, dtype=np.float32)
    aux[:, B] = tok.view(np.float32)
    in_maps = []
    for k in range(N_CORES):
        sl = slice(k * HSH, (k + 1) * HSH)
        # gate columns of Wx for this core: i, g, o slices (f unused: c0 == 0)
        wx_k = np.concatenate(
            [Wx[:, 0 * H:1 * H][:, sl], Wx[:, 2 * H:3 * H][:, sl],
             Wx[:, 3 * H:4 * H][:, sl]], axis=1)
        wx_k = np.ascontiguousarray(wx_k.reshape(KCH, 128, G3))
        bgp_k = np.empty((1, B + G3), np.float32)
        bgp_k[0, :B] = 1.0
        bgp_k[0, B:B + HSH] = b[0 * H:1 * H][sl]
        bgp_k[0, B + HSH:B + 2 * HSH] = b[2 * H:3 * H][sl]
        bgp_k[0, B + 2 * HSH:] = b[3 * H:4 * H][sl]
        in_maps.append({"emb": embedding, "wx": wx_k, "aux": aux, "bgp": bgp_k})
    return in_maps


def _unpack_results(results):
    c = np.empty((B, H), np.float32)
    h = np.empty((B, H), np.float32)
    for k in range(N_CORES):
        sl = slice(k * HSH, (k + 1) * HSH)
        yk = results[k]["y"]
        c[:, sl] = yk[:B]
        h[:, sl] = yk[B:]
    return c, h


def _run_t1(inputs, embedding, Wx, b):
    from concourse.bass_utils import run_bass_kernel_spmd

    if "t1" not in _cache:
        _cache["t1"] = _build_t1_program()
    nc = _cache["t1"]
    in_maps = _make_in_maps(inputs, embedding, Wx, b)
    res = run_bass_kernel_spmd(nc, in_maps, core_ids=list(range(N_CORES)))
    return _unpack_results(res.results)


def kernel(inputs, embedding, Wx, Wh, b):
    inputs = np.asarray(inputs)
    embedding = np.asarray(embedding, dtype=np.float32)
    Wx = np.asarray(Wx, dtype=np.float32)
    Wh = np.asarray(Wh, dtype=np.float32)
    b = np.asarray(b, dtype=np.float32)

    # Exact host-side computation of how many scan steps can change state:
    # sequence bb freezes forever after its first step with
    # embedding[token, EOS_ID] != 0.
    eos = np.zeros((inputs.shape[0],), bool)
    T = 0
    for t in range(inputs.shape[1]):
        eos |= embedding[inputs[:, t], EOS_ID] != 0
        T = t + 1
        if eos.all():
            break

    if T == 1:
        return _run_t1(inputs, embedding, Wx, b)
    # Probability-zero fallback (an embedding value exactly 0.0 at EOS_ID).
    return _lstm_numpy(inputs, embedding, Wx, Wh, b)


# revision 7
# speedup vs baseline: 1.0385x; 1.0089x over previous
"""Trainium2 Bass kernel for nn_Encoder (embedding -> LSTM scan with EOS
state-freezing, returns final (c, h) carry).

Key structural fact: the reference's EOS flag for a sequence is set from
``x[:, EOS_ID].astype(bool)`` where ``x`` is the *float* embedding row of the
current token.  A sequence's state therefore freezes permanently after the
first step whose token embedding has a nonzero feature at column EOS_ID.  The
host computes the exact number of scan steps ``T`` after which every
sequence is frozen (for randn-filled embeddings T == 1 with probability 1)
and the device only has to run those T steps.  For T == 1 the step
simplifies exactly (no approximation): h0 == c0 == 0, so the Wh matmul and
the forget gate contribute exactly nothing:

    gates = x0 @ Wx + b
    c = sigmoid(gates_i) * tanh(gates_g)
    h = sigmoid(gates_o) * tanh(c)

Sharding: the hidden dimension (and with it the i/g/o gate columns of Wx) is
split across the 8 cores, 64 hidden units each.  Each core gathers the 64
first-token embedding rows from the (replicated) table, computes its
[64 batch x 64 hidden] chunk of c and h, and the host concatenates the
chunks into the full [64, 512] outputs.

Device program per core (batch-major gate layout):
  aux DMA (identity + token ids)        [scalar HWDGE queue]
  bias/ones DMA                         [scalar HWDGE queue]
  Wx gate-column shard DMA              [sync HWDGE queue]
  indirect gather of 64 embedding rows, split in two column halves [SWDGE]
  4x PE transpose -> x^T chunks [128, 64]
  gates[64B, 192] = ones^T @ bias  +  sum_c x^T_c^T @ Wx_c   (PSUM accum)
  sigmoid/tanh/mul -> c rows [0:64], h rows [64:128] of one SBUF tile
  one output DMA [128, 64]
"""

import numpy as np

B, S, V, E, H = 64, 512, 32000, 512, 512
EOS_ID = 1
N_CORES = 8
HSH = H // N_CORES  # hidden slice per core: 64
G3 = 3 * HSH        # i/g/o gate columns per core: 192
KCH = E // 128      # contraction chunks: 4

_cache = {}


def _sigmoid(x):
    return 1.0 / (1.0 + np.exp(-x))


def _lstm_numpy(inputs, embedding, Wx, Wh, b):
    """Faithful float32 fallback for the (probability ~0) case where not all
    sequences hit EOS on the first step."""
    Bn = inputs.shape[0]
    c = np.zeros((Bn, H), np.float32)
    h = np.zeros((Bn, H), np.float32)
    eos = np.zeros((Bn,), bool)
    for t in range(inputs.shape[1]):
        x = embedding[inputs[:, t]]
        g = x @ Wx + h @ Wh + b
        gi, gf, gg, go = np.split(g, 4, axis=1)
        new_c = _sigmoid(gf) * c + _sigmoid(gi) * np.tanh(gg)
        new_h = _sigmoid(go) * np.tanh(new_c)
        keep = eos[:, None]
        c = np.where(keep, c, new_c)
        h = np.where(keep, h, new_h)
        eos |= embedding[inputs[:, t], EOS_ID] != 0
        if eos.all():
            break
    return c, h


def _build_t1_program():
    """One-step LSTM cell, gate-column sharded, batch-major gates."""
    import concourse.bacc as bacc
    import concourse.bass as bass
    import concourse.mybir as mybir
    import concourse.tile as tile

    f32 = mybir.dt.float32
    nc = bacc.Bacc("TRN2", target_bir_lowering=False, debug=False,
                   num_devices=N_CORES)

    emb = nc.declare_dram_parameter("emb", [V, E], f32, isOutput=False)
    # Wx gate columns for this core, K-chunk major: [KCH, 128, 192]
    wx = nc.declare_dram_parameter("wx", [KCH, 128, G3], f32, isOutput=False)
    # identity (cols 0:64) + first-token ids as int32 bit pattern (col 64)
    aux = nc.declare_dram_parameter("aux", [B, B + 1], f32, isOutput=False)
    # i/g/o bias slices replicated across the batch partitions
    bgp = nc.declare_dram_parameter("bgp", [B, G3], f32, isOutput=False)
    # rows 0:64 = c chunk [batch, hid], rows 64:128 = h chunk
    y = nc.declare_dram_parameter("y", [2 * B, HSH], f32, isOutput=True)

    with tile.TileContext(nc) as tc:
        with (
            tc.tile_pool(name="sbuf", bufs=1) as sb,
            tc.tile_pool(name="psum", bufs=1, space="PSUM") as ps,
        ):
            # Critical path first: aux gates the gather and the transposes.
            aux_sb = sb.tile([B, B + 1], f32, tag="aux")
            nc.scalar.dma_start(aux_sb[:], aux[:])
            bgp_sb = sb.tile([B, G3], f32, tag="bgp")
            nc.scalar.dma_start(bgp_sb[:], bgp[:])
            wx_sb = sb.tile([128, KCH, G3], f32, tag="wx")
            nc.sync.dma_start(wx_sb[:], wx.ap().rearrange("c p m -> p c m"))

            # Preload the bias into the gates PSUM tile; the matmuls then
            # accumulate on top (start=False) so the bias costs no PE time
            # and no tail instruction.
            gp = ps.tile([B, G3], f32, tag="gates")
            nc.vector.tensor_copy(gp[:], bgp_sb[:])

            tok_ap = aux_sb[:, B:B + 1].bitcast(mybir.dt.int32)
            iden_ap = aux_sb[:, 0:B]

            # Gather the 64 first-token embedding rows, split in two column
            # halves so the first transposes can start while the second half
            # is still in flight.
            x_sb = sb.tile([B, E], f32, tag="x")
            half = E // 2
            for hh in range(2):
                nc.gpsimd.indirect_dma_start(
                    out=x_sb[:, hh * half:(hh + 1) * half],
                    out_offset=None,
                    in_=emb[:],
                    in_offset=bass.IndirectOffsetOnAxis(ap=tok_ap, axis=0),
                    element_offset=hh * half,
                )

            # Transpose to [E, B] in 4 chunks of 128 partitions.
            xt_sb = sb.tile([128, KCH, B], f32, tag="xt")
            for c in range(KCH):
                tp = ps.tile([128, B], f32, tag=f"tp{c}")
                nc.tensor.transpose(tp[:], x_sb[:, c * 128:(c + 1) * 128],
                                    iden_ap)
                nc.vector.tensor_copy(xt_sb[:, c, :], tp[:])

            # gates [64 batch, 192] = bias + sum_c xt_c^T @ wx_c
            for c in range(KCH):
                nc.tensor.matmul(gp[:], lhsT=xt_sb[:, c, :],
                                 rhs=wx_sb[:, c, :], start=False,
                                 stop=(c == KCH - 1))

            Act = mybir.ActivationFunctionType
            out_sb = sb.tile([2 * B, HSH], f32, tag="out")
            sig_i = sb.tile([B, HSH], f32, tag="sig_i")
            nc.scalar.activation(sig_i[:], gp[:, 0:HSH], Act.Sigmoid)
            tanh_g = sb.tile([B, HSH], f32, tag="tanh_g")
            nc.scalar.activation(tanh_g[:], gp[:, HSH:2 * HSH], Act.Tanh)
            # c chunk -> rows 0:64 of the output tile
            nc.vector.tensor_mul(out_sb[0:B, :], sig_i[:], tanh_g[:])

            # h = sigmoid(o) * tanh(c), computed in partitions 64:128 so the
            # output leaves in a single DMA.
            so_tc = sb.tile([2 * B, HSH], f32, tag="so_tc")
            nc.scalar.activation(so_tc[B:2 * B, :], gp[:, 2 * HSH:G3],
                                 Act.Sigmoid)
            tc2 = sb.tile([2 * B, HSH], f32, tag="tc2")
            nc.scalar.activation(tc2[B:2 * B, :], out_sb[0:B, :], Act.Tanh)
            nc.vector.tensor_mul(out_sb[B:2 * B, :], so_tc[B:2 * B, :],
                                 tc2[B:2 * B, :])

            nc.sync.dma_start(y[:], out_sb[:])

    nc.compile()
    return nc


def _make_in_maps(inputs, embedding, Wx, b):
    tok = inputs[:, 0].astype(np.int32)
    aux = np.zeros((B, B + 1), np.float32)
    aux[:, :B] = np.eye(B, dtype=np.float32)
    aux[:, B] = tok.view(np.float32)
    in_maps = []
    for k in range(N_CORES):
        sl = slice(k * HSH, (k + 1) * HSH)
        # gate columns of Wx for this core: i, g, o slices (f unused: c0 == 0)
        wx_k = np.concatenate(
            [Wx[:, 0 * H:1 * H][:, sl], Wx[:, 2 * H:3 * H][:, sl],
             Wx[:, 3 * H:4 * H][:, sl]], axis=1)
        wx_k = np.ascontiguousarray(wx_k.reshape(KCH, 128, G3))
        brow = np.concatenate(
            [b[0 * H:1 * H][sl], b[2 * H:3 * H][sl], b[3 * H:4 * H][sl]])
        bgp_k = np.ascontiguousarray(
            np.broadcast_to(brow.astype(np.float32), (B, G3)))
        in_maps.append({"emb": embedding, "wx": wx_k, "aux": aux, "bgp": bgp_k})
    return in_maps


def _unpack_results(results):
    c = np.empty((B, H), np.float32)
    h = np.empty((B, H), np.float32)
    for k in range(N_CORES):
        sl = slice(k * HSH, (k + 1) * HSH)
        yk = results[k]["y"]
        c[:, sl] = yk[:B]
        h[:, sl] = yk[B:]
    return c, h


def _run_t1(inputs, embedding, Wx, b):
    from concourse.bass_utils import run_bass_kernel_spmd

    if "t1" not in _cache:
        _cache["t1"] = _build_t1_program()
    nc = _cache["t1"]
    in_maps = _make_in_maps(inputs, embedding, Wx, b)
    res = run_bass_kernel_spmd(nc, in_maps, core_ids=list(range(N_CORES)))
    return _unpack_results(res.results)


def kernel(inputs, embedding, Wx, Wh, b):
    inputs = np.asarray(inputs)
    embedding = np.asarray(embedding, dtype=np.float32)
    Wx = np.asarray(Wx, dtype=np.float32)
    Wh = np.asarray(Wh, dtype=np.float32)
    b = np.asarray(b, dtype=np.float32)

    # Exact host-side computation of how many scan steps can change state:
    # sequence bb freezes forever after its first step with
    # embedding[token, EOS_ID] != 0.
    eos = np.zeros((inputs.shape[0],), bool)
    T = 0
    for t in range(inputs.shape[1]):
        eos |= embedding[inputs[:, t], EOS_ID] != 0
        T = t + 1
        if eos.all():
            break

    if T == 1:
        return _run_t1(inputs, embedding, Wx, b)
    # Probability-zero fallback (an embedding value exactly 0.0 at EOS_ID).
    return _lstm_numpy(inputs, embedding, Wx, Wh, b)


# revision 11
# speedup vs baseline: 1.0706x; 1.0308x over previous
"""Trainium2 Bass kernel for nn_Encoder (embedding -> LSTM scan with EOS
state-freezing, returns final (c, h) carry).

Key structural fact: the reference's EOS flag for a sequence is set from
``x[:, EOS_ID].astype(bool)`` where ``x`` is the *float* embedding row of the
current token.  A sequence's state therefore freezes permanently after the
first step whose token embedding has a nonzero feature at column EOS_ID.  The
host computes the exact number of scan steps ``T`` after which every
sequence is frozen (for randn-filled embeddings T == 1 with probability 1)
and the device only has to run those T steps.  For T == 1 the step
simplifies exactly (no approximation): h0 == c0 == 0, so the Wh matmul and
the forget gate contribute exactly nothing:

    gates = x0 @ Wx + b
    c = sigmoid(gates_i) * tanh(gates_g)
    h = sigmoid(gates_o) * tanh(c)

Sharding: the hidden dimension (and with it the i/g/o gate columns of Wx) is
split across the 8 cores, 64 hidden units each.  Each core gathers the 64
first-token embedding rows from the (replicated) table, computes its
[64 batch x 64 hidden] chunk of c and h, and the host concatenates the
chunks into the full [64, 512] outputs.

Device program per core (batch-major gate layout):
  aux DMA (identity + token ids)        [scalar HWDGE queue]
  bias/ones DMA                         [scalar HWDGE queue]
  Wx gate-column shard DMA              [sync HWDGE queue]
  indirect gather of 64 embedding rows, split in two column halves [SWDGE]
  4x PE transpose -> x^T chunks [128, 64]
  gates[64B, 192] = ones^T @ bias  +  sum_c x^T_c^T @ Wx_c   (PSUM accum)
  sigmoid/tanh/mul -> c rows [0:64], h rows [64:128] of one SBUF tile
  one output DMA [128, 64]
"""

import numpy as np

B, S, V, E, H = 64, 512, 32000, 512, 512
EOS_ID = 1
N_CORES = 8
HSH = H // N_CORES  # hidden slice per core: 64
G3 = 3 * HSH        # i/g/o gate columns per core: 192
KCH = E // 128      # contraction chunks: 4

_cache = {}


def _sigmoid(x):
    return 1.0 / (1.0 + np.exp(-x))


def _lstm_numpy(inputs, embedding, Wx, Wh, b):
    """Faithful float32 fallback for the (probability ~0) case where not all
    sequences hit EOS on the first step."""
    Bn = inputs.shape[0]
    c = np.zeros((Bn, H), np.float32)
    h = np.zeros((Bn, H), np.float32)
    eos = np.zeros((Bn,), bool)
    for t in range(inputs.shape[1]):
        x = embedding[inputs[:, t]]
        g = x @ Wx + h @ Wh + b
        gi, gf, gg, go = np.split(g, 4, axis=1)
        new_c = _sigmoid(gf) * c + _sigmoid(gi) * np.tanh(gg)
        new_h = _sigmoid(go) * np.tanh(new_c)
        keep = eos[:, None]
        c = np.where(keep, c, new_c)
        h = np.where(keep, h, new_h)
        eos |= embedding[inputs[:, t], EOS_ID] != 0
        if eos.all():
            break
    return c, h


def _build_t1_program():
    """One-step LSTM cell, gate-column sharded, batch-major gates."""
    import concourse.bacc as bacc
    import concourse.bass as bass
    import concourse.mybir as mybir
    import concourse.tile as tile

    f32 = mybir.dt.float32
    nc = bacc.Bacc("TRN2", target_bir_lowering=False, debug=False,
                   num_devices=N_CORES)

    emb = nc.declare_dram_parameter("emb", [V, E], f32, isOutput=False)
    # Wx gate columns for this core, K-chunk major: [KCH, 128, 192]
    wx = nc.declare_dram_parameter("wx", [KCH, 128, G3], f32, isOutput=False)
    # first-token ids as int32 bit pattern
    tok = nc.declare_dram_parameter("tok", [B, 1], f32, isOutput=False)
    iden = nc.declare_dram_parameter("iden", [B, B], f32, isOutput=False)
    # i/g/o bias slices replicated across the batch partitions
    bgp = nc.declare_dram_parameter("bgp", [B, G3], f32, isOutput=False)
    # rows 0:64 = c chunk [batch, hid], rows 64:128 = h chunk
    y = nc.declare_dram_parameter("y", [2 * B, HSH], f32, isOutput=True)

    with tile.TileContext(nc) as tc:
        with (
            tc.tile_pool(name="sbuf", bufs=1) as sb,
            tc.tile_pool(name="psum", bufs=1, space="PSUM") as ps,
        ):
            # Critical path first: the token DMA gates the gather.
            tok_sb = sb.tile([B, 1], f32, tag="tok")
            nc.sync.dma_start(tok_sb[:], tok[:])
            wx_sb = sb.tile([128, KCH, G3], f32, tag="wx")
            nc.sync.dma_start(wx_sb[:], wx.ap().rearrange("c p m -> p c m"))
            iden_sb = sb.tile([B, B], f32, tag="iden")
            nc.scalar.dma_start(iden_sb[:], iden[:])
            bgp_sb = sb.tile([B, G3], f32, tag="bgp")
            nc.scalar.dma_start(bgp_sb[:], bgp[:])

            # PE warm-up: ~3.4us of dummy bf16 matmuls on scratch flips the
            # HAM clock gate to 2.4 GHz before the real matmuls arrive.
            # No input dependencies: runs while the gather is in flight.
            bf16 = mybir.dt.bfloat16
            warm_sb = sb.tile([128, 512], bf16, tag="warm")
            nc.gpsimd.memset(warm_sb[:], 0.0)
            warm_ps = ps.tile([128, 512], f32, tag="warm_ps")
            for _ in range(8):
                nc.tensor.matmul(warm_ps[:], lhsT=warm_sb[:, 0:128],
                                 rhs=warm_sb[:], start=True, stop=True)

            # Preload the bias into the gates PSUM tile; the matmuls then
            # accumulate on top (start=False) so the bias costs no PE time
            # and no tail instruction.
            gp = ps.tile([B, G3], f32, tag="gates")
            nc.vector.tensor_copy(gp[:], bgp_sb[:])

            tok_ap = tok_sb[:, 0:1].bitcast(mybir.dt.int32)
            iden_ap = iden_sb[:]

            # Gather the 64 first-token embedding rows.
            x_sb = sb.tile([B, E], f32, tag="x")
            nc.gpsimd.indirect_dma_start(
                out=x_sb[:],
                out_offset=None,
                in_=emb[:],
                in_offset=bass.IndirectOffsetOnAxis(ap=tok_ap, axis=0),
            )

            # Transpose to [E, B] in 4 chunks of 128 partitions.
            xt_sb = sb.tile([128, KCH, B], f32, tag="xt")
            for c in range(KCH):
                tp = ps.tile([128, B], f32, tag=f"tp{c}")
                nc.tensor.transpose(tp[:], x_sb[:, c * 128:(c + 1) * 128],
                                    iden_ap)
                nc.vector.tensor_copy(xt_sb[:, c, :], tp[:])

            # gates [64 batch, 192] = bias + sum_c xt_c^T @ wx_c
            for c in range(KCH):
                nc.tensor.matmul(gp[:], lhsT=xt_sb[:, c, :],
                                 rhs=wx_sb[:, c, :], start=False,
                                 stop=(c == KCH - 1))

            Act = mybir.ActivationFunctionType
            out_sb = sb.tile([2 * B, HSH], f32, tag="out")
            sig_i = sb.tile([B, HSH], f32, tag="sig_i")
            nc.scalar.activation(sig_i[:], gp[:, 0:HSH], Act.Sigmoid)
            tanh_g = sb.tile([B, HSH], f32, tag="tanh_g")
            nc.scalar.activation(tanh_g[:], gp[:, HSH:2 * HSH], Act.Tanh)
            # c chunk -> rows 0:64 of the output tile
            nc.vector.tensor_mul(out_sb[0:B, :], sig_i[:], tanh_g[:])

            # h = sigmoid(o) * tanh(c), computed in partitions 64:128 so the
            # output leaves in a single DMA.
            so_tc = sb.tile([2 * B, HSH], f32, tag="so_tc")
            nc.scalar.activation(so_tc[B:2 * B, :], gp[:, 2 * HSH:G3],
                                 Act.Sigmoid)
            tc2 = sb.tile([2 * B, HSH], f32, tag="tc2")
            nc.scalar.activation(tc2[B:2 * B, :], out_sb[0:B, :], Act.Tanh)
            nc.vector.tensor_mul(out_sb[B:2 * B, :], so_tc[B:2 * B, :],
                                 tc2[B:2 * B, :])

            nc.sync.dma_start(y[:], out_sb[:])

    nc.compile()
    return nc


def _make_in_maps(inputs, embedding, Wx, b):
    tok = np.ascontiguousarray(
        inputs[:, 0].astype(np.int32).view(np.float32).reshape(B, 1))
    iden = np.eye(B, dtype=np.float32)
    in_maps = []
    for k in range(N_CORES):
        sl = slice(k * HSH, (k + 1) * HSH)
        # gate columns of Wx for this core: i, g, o slices (f unused: c0 == 0)
        wx_k = np.concatenate(
            [Wx[:, 0 * H:1 * H][:, sl], Wx[:, 2 * H:3 * H][:, sl],
             Wx[:, 3 * H:4 * H][:, sl]], axis=1)
        wx_k = np.ascontiguousarray(wx_k.reshape(KCH, 128, G3))
        brow = np.concatenate(
            [b[0 * H:1 * H][sl], b[2 * H:3 * H][sl], b[3 * H:4 * H][sl]])
        bgp_k = np.ascontiguousarray(
            np.broadcast_to(brow.astype(np.float32), (B, G3)))
        in_maps.append({"emb": embedding, "wx": wx_k, "tok": tok, "iden": iden,
                        "bgp": bgp_k})
    return in_maps


def _unpack_results(results):
    c = np.empty((B, H), np.float32)
    h = np.empty((B, H), np.float32)
    for k in range(N_CORES):
        sl = slice(k * HSH, (k + 1) * HSH)
        yk = results[k]["y"]
        c[:, sl] = yk[:B]
        h[:, sl] = yk[B:]
    return c, h


def _run_t1(inputs, embedding, Wx, b):
    from concourse.bass_utils import run_bass_kernel_spmd

    if "t1" not in _cache:
        _cache["t1"] = _build_t1_program()
    nc = _cache["t1"]
    in_maps = _make_in_maps(inputs, embedding, Wx, b)
    res = run_bass_kernel_spmd(nc, in_maps, core_ids=list(range(N_CORES)))
    return _unpack_results(res.results)


def kernel(inputs, embedding, Wx, Wh, b):
    inputs = np.asarray(inputs)
    embedding = np.asarray(embedding, dtype=np.float32)
    Wx = np.asarray(Wx, dtype=np.float32)
    Wh = np.asarray(Wh, dtype=np.float32)
    b = np.asarray(b, dtype=np.float32)

    # Exact host-side computation of how many scan steps can change state:
    # sequence bb freezes forever after its first step with
    # embedding[token, EOS_ID] != 0.
    eos = np.zeros((inputs.shape[0],), bool)
    T = 0
    for t in range(inputs.shape[1]):
        eos |= embedding[inputs[:, t], EOS_ID] != 0
        T = t + 1
        if eos.all():
            break

    if T == 1:
        return _run_t1(inputs, embedding, Wx, b)
    # Probability-zero fallback (an embedding value exactly 0.0 at EOS_ID).
    return _lstm_numpy(inputs, embedding, Wx, Wh, b)


# revision 12
# speedup vs baseline: 1.1078x; 1.0348x over previous
"""Trainium2 Bass kernel for nn_Encoder (embedding -> LSTM scan with EOS
state-freezing, returns final (c, h) carry).

Key structural fact: the reference's EOS flag for a sequence is set from
``x[:, EOS_ID].astype(bool)`` where ``x`` is the *float* embedding row of the
current token.  A sequence's state therefore freezes permanently after the
first step whose token embedding has a nonzero feature at column EOS_ID.  The
host computes the exact number of scan steps ``T`` after which every
sequence is frozen (for randn-filled embeddings T == 1 with probability 1)
and the device only has to run those T steps.  For T == 1 the step
simplifies exactly (no approximation): h0 == c0 == 0, so the Wh matmul and
the forget gate contribute exactly nothing:

    gates = x0 @ Wx + b
    c = sigmoid(gates_i) * tanh(gates_g)
    h = sigmoid(gates_o) * tanh(c)

Sharding: the hidden dimension (and with it the i/g/o gate columns of Wx) is
split across the 8 cores, 64 hidden units each.  Each core gathers the 64
first-token embedding rows from the (replicated) table, computes its
[64 batch x 64 hidden] chunk of c and h, and the host concatenates the
chunks into the full [64, 512] outputs.

Device program per core (batch-major gate layout):
  aux DMA (identity + token ids)        [scalar HWDGE queue]
  bias/ones DMA                         [scalar HWDGE queue]
  Wx gate-column shard DMA              [sync HWDGE queue]
  indirect gather of 64 embedding rows, split in two column halves [SWDGE]
  4x PE transpose -> x^T chunks [128, 64]
  gates[64B, 192] = ones^T @ bias  +  sum_c x^T_c^T @ Wx_c   (PSUM accum)
  sigmoid/tanh/mul -> c rows [0:64], h rows [64:128] of one SBUF tile
  one output DMA [128, 64]
"""

import numpy as np

B, S, V, E, H = 64, 512, 32000, 512, 512
EOS_ID = 1
N_CORES = 8
HSH = H // N_CORES  # hidden slice per core: 64
G3 = 3 * HSH        # i/g/o gate columns per core: 192
KCH = E // 128      # contraction chunks: 4

_cache = {}


def _sigmoid(x):
    return 1.0 / (1.0 + np.exp(-x))


def _lstm_numpy(inputs, embedding, Wx, Wh, b):
    """Faithful float32 fallback for the (probability ~0) case where not all
    sequences hit EOS on the first step."""
    Bn = inputs.shape[0]
    c = np.zeros((Bn, H), np.float32)
    h = np.zeros((Bn, H), np.float32)
    eos = np.zeros((Bn,), bool)
    for t in range(inputs.shape[1]):
        x = embedding[inputs[:, t]]
        g = x @ Wx + h @ Wh + b
        gi, gf, gg, go = np.split(g, 4, axis=1)
        new_c = _sigmoid(gf) * c + _sigmoid(gi) * np.tanh(gg)
        new_h = _sigmoid(go) * np.tanh(new_c)
        keep = eos[:, None]
        c = np.where(keep, c, new_c)
        h = np.where(keep, h, new_h)
        eos |= embedding[inputs[:, t], EOS_ID] != 0
        if eos.all():
            break
    return c, h


def _build_t1_program():
    """One-step LSTM cell, gate-column sharded, batch-major gates."""
    import concourse.bacc as bacc
    import concourse.bass as bass
    import concourse.mybir as mybir
    import concourse.tile as tile

    f32 = mybir.dt.float32
    nc = bacc.Bacc("TRN2", target_bir_lowering=False, debug=False,
                   num_devices=N_CORES)

    emb = nc.declare_dram_parameter("emb", [V, E], f32, isOutput=False)
    # Wx gate columns for this core, K-chunk major: [KCH, 128, 192]
    wx = nc.declare_dram_parameter("wx", [KCH, 128, G3], f32, isOutput=False)
    # first-token ids as int32 bit pattern
    tok = nc.declare_dram_parameter("tok", [B, 1], f32, isOutput=False)
    iden = nc.declare_dram_parameter("iden", [B, B], f32, isOutput=False)
    # i/g/o bias slices replicated across the batch partitions
    bgp = nc.declare_dram_parameter("bgp", [B, G3], f32, isOutput=False)
    # rows 0:64 = c chunk [batch, hid], rows 64:128 = h chunk
    y = nc.declare_dram_parameter("y", [2 * B, HSH], f32, isOutput=True)

    with tile.TileContext(nc) as tc:
        with (
            tc.tile_pool(name="sbuf", bufs=1) as sb,
            tc.tile_pool(name="psum", bufs=1, space="PSUM") as ps,
        ):
            # Critical path first: the token DMA gates the gather.
            tok_sb = sb.tile([B, 1], f32, tag="tok")
            nc.sync.dma_start(tok_sb[:], tok[:])
            wx_sb = sb.tile([128, KCH, G3], f32, tag="wx")
            nc.sync.dma_start(wx_sb[:], wx.ap().rearrange("c p m -> p c m"))
            iden_sb = sb.tile([B, B], f32, tag="iden")
            nc.scalar.dma_start(iden_sb[:], iden[:])
            bgp_sb = sb.tile([B, G3], f32, tag="bgp")
            nc.scalar.dma_start(bgp_sb[:], bgp[:])

            # PE warm-up: ~3.4us of dummy bf16 matmuls on scratch flips the
            # HAM clock gate to 2.4 GHz before the real matmuls arrive.
            # No input dependencies: runs while the gather is in flight.
            bf16 = mybir.dt.bfloat16
            warm_sb = sb.tile([128, 512], bf16, tag="warm")
            nc.gpsimd.memset(warm_sb[:], 0.0)
            warm_ps = ps.tile([128, 512], f32, tag="warm_ps")
            for _ in range(9):
                nc.tensor.matmul(warm_ps[:], lhsT=warm_sb[:, 0:128],
                                 rhs=warm_sb[:], start=True, stop=True)

            # Preload the bias into the gates PSUM tile; the matmuls then
            # accumulate on top (start=False) so the bias costs no PE time
            # and no tail instruction.
            gp = ps.tile([B, G3], f32, tag="gates")
            nc.vector.tensor_copy(gp[:], bgp_sb[:])

            tok_ap = tok_sb[:, 0:1].bitcast(mybir.dt.int32)
            iden_ap = iden_sb[:]

            # Gather the 64 first-token embedding rows.
            x_sb = sb.tile([B, E], f32, tag="x")
            nc.gpsimd.indirect_dma_start(
                out=x_sb[:],
                out_offset=None,
                in_=emb[:],
                in_offset=bass.IndirectOffsetOnAxis(ap=tok_ap, axis=0),
            )

            # Transpose to [E, B] in 4 chunks of 128 partitions.
            xt_sb = sb.tile([128, KCH, B], f32, tag="xt")
            for c in range(KCH):
                tp = ps.tile([128, B], f32, tag=f"tp{c}")
                nc.tensor.transpose(tp[:], x_sb[:, c * 128:(c + 1) * 128],
                                    iden_ap)
                nc.vector.tensor_copy(xt_sb[:, c, :], tp[:])

            # gates [64 batch, 192] = bias + sum_c xt_c^T @ wx_c
            for c in range(KCH):
                nc.tensor.matmul(gp[:], lhsT=xt_sb[:, c, :],
                                 rhs=wx_sb[:, c, :], start=False,
                                 stop=(c == KCH - 1))

            Act = mybir.ActivationFunctionType
            out_sb = sb.tile([2 * B, HSH], f32, tag="out")
            sig_i = sb.tile([B, HSH], f32, tag="sig_i")
            nc.scalar.activation(sig_i[:], gp[:, 0:HSH], Act.Sigmoid)
            tanh_g = sb.tile([B, HSH], f32, tag="tanh_g")
            nc.scalar.activation(tanh_g[:], gp[:, HSH:2 * HSH], Act.Tanh)
            # c chunk -> rows 0:64 of the output tile
            nc.vector.tensor_mul(out_sb[0:B, :], sig_i[:], tanh_g[:])

            # h = sigmoid(o) * tanh(c), computed in partitions 64:128 so the
            # output leaves in a single DMA.
            so_tc = sb.tile([2 * B, HSH], f32, tag="so_tc")
            nc.scalar.activation(so_tc[B:2 * B, :], gp[:, 2 * HSH:G3],
                                 Act.Sigmoid)
            tc2 = sb.tile([2 * B, HSH], f32, tag="tc2")
            nc.scalar.activation(tc2[B:2 * B, :], out_sb[0:B, :], Act.Tanh)
            nc.vector.tensor_mul(out_sb[B:2 * B, :], so_tc[B:2 * B, :],
                                 tc2[B:2 * B, :])

            nc.sync.dma_start(y[:], out_sb[:])

    nc.compile()
    return nc


def _make_in_maps(inputs, embedding, Wx, b):
    tok = np.ascontiguousarray(
        inputs[:, 0].astype(np.int32).view(np.float32).reshape(B, 1))
    iden = np.eye(B, dtype=np.float32)
    in_maps = []
    for k in range(N_CORES):
        sl = slice(k * HSH, (k + 1) * HSH)
        # gate columns of Wx for this core: i, g, o slices (f unused: c0 == 0)
        wx_k = np.concatenate(
            [Wx[:, 0 * H:1 * H][:, sl], Wx[:, 2 * H:3 * H][:, sl],
             Wx[:, 3 * H:4 * H][:, sl]], axis=1)
        wx_k = np.ascontiguousarray(wx_k.reshape(KCH, 128, G3))
        brow = np.concatenate(
            [b[0 * H:1 * H][sl], b[2 * H:3 * H][sl], b[3 * H:4 * H][sl]])
        bgp_k = np.ascontiguousarray(
            np.broadcast_to(brow.astype(np.float32), (B, G3)))
        in_maps.append({"emb": embedding, "wx": wx_k, "tok": tok, "iden": iden,
                        "bgp": bgp_k})
    return in_maps


def _unpack_results(results):
    c = np.empty((B, H), np.float32)
    h = np.empty((B, H), np.float32)
    for k in range(N_CORES):
        sl = slice(k * HSH, (k + 1) * HSH)
        yk = results[k]["y"]
        c[:, sl] = yk[:B]
        h[:, sl] = yk[B:]
    return c, h


def _run_t1(inputs, embedding, Wx, b):
    from concourse.bass_utils import run_bass_kernel_spmd

    if "t1" not in _cache:
        _cache["t1"] = _build_t1_program()
    nc = _cache["t1"]
    in_maps = _make_in_maps(inputs, embedding, Wx, b)
    res = run_bass_kernel_spmd(nc, in_maps, core_ids=list(range(N_CORES)))
    return _unpack_results(res.results)


def kernel(inputs, embedding, Wx, Wh, b):
    inputs = np.asarray(inputs)
    embedding = np.asarray(embedding, dtype=np.float32)
    Wx = np.asarray(Wx, dtype=np.float32)
    Wh = np.asarray(Wh, dtype=np.float32)
    b = np.asarray(b, dtype=np.float32)

    # Exact host-side computation of how many scan steps can change state:
    # sequence bb freezes forever after its first step with
    # embedding[token, EOS_ID] != 0.
    eos = np.zeros((inputs.shape[0],), bool)
    T = 0
    for t in range(inputs.shape[1]):
        eos |= embedding[inputs[:, t], EOS_ID] != 0
        T = t + 1
        if eos.all():
            break

    if T == 1:
        return _run_t1(inputs, embedding, Wx, b)
    # Probability-zero fallback (an embedding value exactly 0.0 at EOS_ID).
    return _lstm_numpy(inputs, embedding, Wx, Wh, b)
